# revision 46
# baseline (speedup 1.0000x reference)
"""Causal multi-head self-attention on 8 Trainium2 NeuronCores.

Sharding: core c <- (batch b = c//2, head-group hg = c%2 of 8 heads).
Data-parallel over B, tensor-parallel over H. Each core computes
  q,k,v = x[b] @ W{q,k,v}[hg-slice].T   (+bias via augmented ones-row)
  per-head causal attention (flash-style, scores never hit HBM)
  y_part = attn_out @ Wo[:, hg-slice].T
Host sums the two partials per batch and adds bo.

On-chip layout (all bf16 in SBUF, fp32 accumulation in PSUM):
  xT   [128, 8, T]   x[b]^T, contraction dim c on partitions (8 chunks)
  qT,kT[128, 4, T]   q^T/k^T, head-dim on partitions (head h -> chunk h//2,
                     partition offset 64*(h%2)); Wq pre-scaled by 1/8 on host
  vP   [128,16,8,65] v tiles [t-part, head, d] with a ones column at d=64
                     (gives the softmax denominator for free in the O matmul)
  S^T tiles [128k, Nq] = kT_blk.T @ qT_blk  -> exp on ACT -> P^T bf16
    diagonal-band tiles stream only the unmasked column range (N = 512-128r)
  O^T' [65, 512] accumulates V'_i.T @ P^T_i over k-blocks; row 64 = sum(exp)
  normalize via DVE reciprocal + DMA row-broadcast + DVE multiply
  AT   [128, 4, T]   normalized attention output, c' on partitions
  y    [T, C] fp32   = AT.T @ WoT (4 c'-chunk accumulation)

v-projection tiles 4..15 and the output projection are emitted as PE
"fillers" interleaved into the attention loop, which is otherwise bound by
ACT's exp throughput.
"""

from collections import deque

import numpy as np
import ml_dtypes

import concourse.bass as bass
import concourse.mybir as mybir
import concourse.tile as tile
from concourse.bass_utils import run_bass_kernel_spmd
from concourse.alu_op_type import AluOpType

B, T, C, H, D = 4, 2048, 1024, 16, 64
NCORES = 8
HPC = H // 2          # heads per core
CP = HPC * D          # channels per core (512)
BF = mybir.dt.bfloat16
F32 = mybir.dt.float32
NP_BF16 = ml_dtypes.bfloat16

TQ = 512              # q-block (free dim, one PSUM bank of fp32)
TK = 128              # k-block (partition dim)
NJ = T // TQ          # 4 q-blocks
NTT = T // TK         # 16 t-tiles


def _split_multiwait(nc, cap=1):
    """The walrus in this container rejects instructions carrying more than
    one sync wait; tile's tail drain waits once per engine/DMA queue. Move
    the extras onto single-wait NOPs inserted just before the instruction."""
    n = 0
    for fn in nc.m.functions:
        for blk in fn.blocks:
            insts = blk.instructions
            out = []
            changed = False
            for ins in insts:
                si = ins.sync_info
                waits = list(si.on_wait) if si is not None else []
                if len(waits) > cap:
                    changed = True
                    for w in waits[cap:]:
                        nop = mybir.InstNoOp(
                            name=f"splitwait-{n}",
                            engine=ins.engine,
                            sync_info=mybir.SyncInfo(on_wait=[w], on_update=[]),
                            bass_nofuse=True,
                        )
                        n += 1
                        out.append(nop)
                    ins.sync_info = mybir.SyncInfo(
                        on_wait=waits[:cap], on_update=list(si.on_update)
                    )
                out.append(ins)
            if changed:
                blk.instructions = out
    return n


def _build_program(aug: bool):
    nc = bass.Bass()
    xT_d = nc.declare_dram_parameter("xT", [128, 8, T], BF, isOutput=False)
    wq_d = nc.declare_dram_parameter("wq", [128, 8, CP], BF, isOutput=False)
    wk_d = nc.declare_dram_parameter("wk", [128, 8, CP], BF, isOutput=False)
    wv_d = nc.declare_dram_parameter("wv", [128, 8, CP], BF, isOutput=False)
    wo_d = nc.declare_dram_parameter("wo", [128, 4, C], BF, isOutput=False)
    mk_d = nc.declare_dram_parameter("masks", [128, TQ], BF, isOutput=False)
    if aug:
        wa_d = nc.declare_dram_parameter("waug", [1, 3, CP], BF, isOutput=False)
    y_d = nc.declare_dram_parameter("y", [T, C], F32, isOutput=True)

    with tile.TileContext(nc) as tc:
        with (
            tc.tile_pool(name="big", bufs=1) as big,
            tc.tile_pool(name="pt", bufs=16) as ppool,
            tc.tile_pool(name="sm", bufs=6) as small,
            tc.tile_pool(name="yo", bufs=4) as ypool,
            tc.tile_pool(name="ps1", bufs=2, space="PSUM") as ps1,
            tc.tile_pool(name="psS", bufs=2, space="PSUM") as psS,
            tc.tile_pool(name="psO", bufs=2, space="PSUM") as psO,
        ):
            # weights first, x chunk-by-chunk, so the first projection
            # matmuls can start as soon as chunk 0 lands
            # spread input loads over the three DMA-capable queues so the
            # first projection matmuls unblock chunk by chunk
            wq = big.tile([128, 8, CP], BF, tag="wq")
            xt = big.tile([128, 8, T], BF, tag="xt")
            for c in range(8):
                nc.gpsimd.dma_start(wq[:, c, :], wq_d[:, c, :])
                eng = nc.sync if c % 2 == 0 else nc.scalar
                eng.dma_start(xt[:, c, :], xT_d[:, c, :])
            wk = big.tile([128, 8, CP], BF, tag="wk")
            nc.gpsimd.dma_start(wk[:], wk_d[:])
            wv = big.tile([128, 8, CP], BF, tag="wv")
            nc.gpsimd.dma_start(wv[:], wv_d[:])
            mks = big.tile([128, TQ], BF, tag="mks")
            nc.gpsimd.dma_start(mks[:], mk_d[:])
            wo = big.tile([128, 4, C], BF, tag="wo")
            nc.gpsimd.dma_start(wo[:], wo_d[:])
            if aug:
                wa = big.tile([1, 3, CP], BF, tag="wa")
                nc.sync.dma_start(wa[:], wa_d[:])
                one = big.tile([1, T], BF, tag="one")
                nc.vector.memset(one[:], 1.0)

            qT = big.tile([128, 4, T], BF, tag="qT")
            kT = big.tile([128, 4, T], BF, tag="kT")
            vP = big.tile([128, NTT, HPC, D + 1], BF, tag="vP")
            AT = big.tile([128, 4, T], BF, tag="AT")
            # softmax-denominator ones column of vP
            nc.vector.memset(vP[:, :, :, D], 1.0)
            # pre-warm ACT's Exp table during the (idle) load phase so the
            # first real exp doesn't pay the table-load mid-pipeline
            warm = small.tile([1, 1], F32, tag="warm", name="warm")
            nc.vector.memset(warm[:], 0.0)
            nc.scalar.activation(
                warm[:], warm[:], mybir.ActivationFunctionType.Exp
            )

            # ---- q/k projection generator for one m-chunk (PE filler) ----
            def g_qk(m):
                for wt, outT, bix in ((wq, qT, 0), (wk, kT, 1)):
                    for tcb in range(NJ):
                        ps = ps1.tile(
                            [128, TQ], F32, tag="ps1", name=f"pqk{bix}_{m}_{tcb}"
                        )
                        for c in range(8):
                            nc.tensor.matmul(
                                ps[:],
                                wt[:, c, 128 * m : 128 * m + 128],
                                xt[:, c, TQ * tcb : TQ * tcb + TQ],
                                start=(c == 0),
                                stop=(c == 7 and not aug),
                            )
                            yield True
                        if aug:
                            nc.tensor.matmul(
                                ps[:],
                                wa[:, bix, 128 * m : 128 * m + 128],
                                one[:, TQ * tcb : TQ * tcb + TQ],
                                start=False,
                                stop=True,
                            )
                            yield True
                        nc.vector.tensor_copy(
                            outT[:, m, TQ * tcb : TQ * tcb + TQ], ps[:]
                        )

            # ---- v projection tile: out [t-part, dv-free], strided into vP ----
            def g_vtile(tt):
                ps = ps1.tile([128, CP], F32, tag="ps1", name=f"vps{tt}")
                for c in range(8):
                    nc.tensor.matmul(
                        ps[:],
                        xt[:, c, TK * tt : TK * tt + TK],
                        wv[:, c, :],
                        start=(c == 0),
                        stop=(c == 7 and not aug),
                    )
                    yield True
                if aug:
                    nc.tensor.matmul(
                        ps[:],
                        one[:, TK * tt : TK * tt + TK],
                        wa[:, 2, :],
                        start=False,
                        stop=True,
                    )
                    yield True
                # [128, 512] -> [128, 8, 64] strided dest (skips ones col)
                nc.vector.tensor_copy(vP[:, tt, :, 0:D], ps[:])

            # heads whose attention output (AT) has been fully emitted, per j
            heads_done = [0] * NJ

            # ---- output projection for one t-tile (PE filler generator) ----
            # cp-chunk cp reads AT rows of heads 2cp/2cp+1 only; spin (yield
            # False) until those heads' normalize has been emitted for this j
            def g_outproj(tt):
                j = tt // 4
                while heads_done[j] < 2:
                    yield False
                for co in range(2):
                    ps = ps1.tile([128, TQ], F32, tag="ps1", name=f"y{tt}_{co}")
                    for cp in range(4):
                        while heads_done[j] < 2 * cp + 2:
                            yield False
                        nc.tensor.matmul(
                            ps[:],
                            AT[:, cp, TK * tt : TK * tt + TK],
                            wo[:, cp, TQ * co : TQ * co + TQ],
                            start=(cp == 0),
                            stop=(cp == 3),
                        )
                        yield True
                    yt = ypool.tile([128, TQ], F32, tag="yo", name=f"yt{tt}_{co}")
                    nc.vector.tensor_copy(yt[:], ps[:])
                    nc.gpsimd.dma_start(
                        y_d[TK * tt : TK * tt + TK, TQ * co : TQ * co + TQ], yt[:]
                    )

            # chain the per-tile output projections into ONE generator so at
            # most one is active (holding one ps1 slot); two spinning holders
            # deadlock the schedule against the v/qk fillers
            def g_outproj_chain(tts):
                for tt in tts:
                    yield from g_outproj(tt)

            qk_gens = [g_qk(m) for m in range(4)]
            v_gens = [g_vtile(tt) for tt in range(NTT)]

            fillers = deque(
                [qk_gens[1], qk_gens[2], qk_gens[3]] + v_gens[4:]
            )

            def drain(n):
                tries = len(fillers)
                while n > 0 and fillers and tries > 0:
                    try:
                        if next(fillers[0]):
                            n -= 1
                            tries = len(fillers)
                        else:
                            fillers.rotate(-1)
                            tries -= 1
                    except StopIteration:
                        fillers.popleft()
                        tries = len(fillers)

            def ensure(g):
                # force-finish a projection generator whose output the next
                # attention block needs (no-op if drains already finished it)
                for _ in g:
                    pass

            # ---- attention: head-pair-outer so exp work starts right after
            # the m=0 projections; later q/k chunks, v tiles and the output
            # projection drip in as PE fillers while ACT chews on exp ----
            ensure(qk_gens[0])
            for p in range(HPC // 2):
                ensure(qk_gens[p])
                if p == 2 and not aug:
                    # hold the output projection back until the late,
                    # ACT-bound pairs need PE filler work (aug path: the
                    # extra bias matmuls shift drain timing into a Tile
                    # scheduling deadlock, so it runs the projection after
                    # attention instead -- see below)
                    fillers.append(g_outproj_chain(range(NTT)))
                for j in range(NJ):
                    for tt in range(4 * j + 4):
                        ensure(v_gens[tt])
                    h1, h2 = 2 * p, 2 * p + 1
                    q1, k1 = qT[0:64, p, :], kT[0:64, p, :]
                    q2, k2 = qT[64:128, p, :], kT[64:128, p, :]
                    ni = 4 * j + 4
                    op1 = psO.tile([D + 1, TQ], F32, tag="opsum", name=f"op{h1}_{j}")
                    op2 = psO.tile([D + 1, TQ], F32, tag="opsum", name=f"op{h2}_{j}")

                    # one S psum (2 banks) and ONE exp for both heads of the
                    # pair at the same (j, i): halves ACT's per-op overhead
                    def s_step(i):
                        r = i - 4 * j
                        f0 = 128 * r if r > 0 else 0
                        n = TQ - f0
                        sp = psS.tile(
                            [128, 2 * TQ], F32, tag="spsum", name=f"sp{j}_{i}"
                        )
                        pt = ppool.tile(
                            [128, 2 * TQ], BF, tag="pt", name=f"pt{j}_{i}"
                        )
                        nc.tensor.matmul(
                            sp[:, 0:n],
                            k1[:, TK * i : TK * i + TK],
                            q1[:, TQ * j + f0 : TQ * (j + 1)],
                            start=True,
                            stop=True,
                        )
                        nc.tensor.matmul(
                            sp[:, TQ : TQ + n],
                            k2[:, TK * i : TK * i + TK],
                            q2[:, TQ * j + f0 : TQ * (j + 1)],
                            start=True,
                            stop=True,
                        )
                        sp_ap = bass.AP(
                            sp.tensor, sp.offset, [list(sp.ap)[0], [TQ, 2], [1, n]]
                        )
                        pt_ap = bass.AP(
                            pt.tensor, pt.offset, [list(pt.ap)[0], [TQ, 2], [1, n]]
                        )
                        nc.scalar.activation(
                            pt_ap, sp_ap, mybir.ActivationFunctionType.Exp
                        )
                        if r >= 0:
                            # only the first 128 columns of a sliced diagonal
                            # tile are triangular; columns >= 128 satisfy
                            # p <= 127 < 128 <= c and need no mask
                            for base in (0, TQ):
                                nc.vector.tensor_tensor(
                                    pt[:, base : base + TK],
                                    pt[:, base : base + TK],
                                    mks[:, 0:TK],
                                    op=AluOpType.mult,
                                )
                        return pt, f0, n

                    def o_step(i, pt, f0, n):
                        nc.tensor.matmul(
                            op1[:, f0 : f0 + n],
                            vP[:, i, h1, :],
                            pt[:, 0:n],
                            start=(i == 0),
                            stop=(i == ni - 1),
                        )
                        nc.tensor.matmul(
                            op2[:, f0 : f0 + n],
                            vP[:, i, h2, :],
                            pt[:, TQ : TQ + n],
                            start=(i == 0),
                            stop=(i == ni - 1),
                        )

                    # software-pipelined: O lags S by one (i covers both heads)
                    pend = deque([s_step(0)])
                    for i in range(1, ni):
                        cur = s_step(i)
                        o_step(i - 1, *pend.popleft())
                        drain(1)
                        pend.append(cur)
                    o_step(ni - 1, *pend.popleft())
                    drain(1)

                    # normalize rows 0..63 by the exp-sum in row 64
                    for hh, po, op in ((h1, 0, op1), (h2, 64, op2)):
                        r_ = small.tile([1, TQ], F32, tag="recip", name=f"rc{hh}_{j}")
                        nc.vector.reciprocal(r_[:], op[D : D + 1, :])
                        bc = small.tile([D, TQ], F32, tag="bc", name=f"bc{hh}_{j}")
                        src_ = r_[0:1, :]
                        bcast_src = bass.AP(
                            src_.tensor,
                            src_.offset,
                            [list(src_.ap)[0], [0, D]] + list(src_.ap)[1:],
                        )
                        nc.sync.dma_start(bc[:], bcast_src)
                        nc.vector.tensor_tensor(
                            AT[po : po + 64, p, TQ * j : TQ * j + TQ],
                            op[0:D, :],
                            bc[:],
                            op=AluOpType.mult,
                        )
                    heads_done[j] += 2

            if aug:
                fillers.append(g_outproj_chain(range(NTT)))
            drain(1 << 30)

    _split_multiwait(nc)
    return nc


_cache = {}


def _get_program(aug: bool):
    if aug not in _cache:
        _cache[aug] = _build_program(aug)
    return _cache[aug]


def _prep_w(Wsl):
    """[512 out, 1024 in] torch-Linear slice -> [128, 8, 512] bf16 (in-dim on
    partitions, chunked)."""
    WT = np.ascontiguousarray(Wsl.T)  # [1024, 512]
    return np.ascontiguousarray(
        WT.reshape(8, 128, CP).transpose(1, 0, 2)
    ).astype(NP_BF16)


def _masks():
    p = np.arange(128)[:, None]
    f = np.arange(TQ)[None, :]
    return (p <= f).astype(NP_BF16)  # [128, 512]


def kernel(x, Wq, bq, Wk, bk, Wv, bv, Wo, bo):
    x = np.asarray(x, np.float32)
    Wq, bq = np.asarray(Wq, np.float32), np.asarray(bq, np.float32)
    Wk, bk = np.asarray(Wk, np.float32), np.asarray(bk, np.float32)
    Wv, bv = np.asarray(Wv, np.float32), np.asarray(bv, np.float32)
    Wo, bo = np.asarray(Wo, np.float32), np.asarray(bo, np.float32)

    aug = bool(np.any(bq) or np.any(bk) or np.any(bv))
    nc = _get_program(aug)

    # fold the 1/sqrt(D) score scale into Wq (and bq)
    Wq8, bq8 = Wq * 0.125, bq * 0.125
    masks = _masks()

    in_maps = []
    for c in range(NCORES):
        b, hg = divmod(c, 2)
        sl = slice(hg * CP, hg * CP + CP)
        xT = np.ascontiguousarray(x[b].T)  # [1024, 2048]
        xTh = np.ascontiguousarray(
            xT.reshape(8, 128, T).transpose(1, 0, 2)
        ).astype(NP_BF16)
        m = {
            "xT": xTh,
            "wq": _prep_w(Wq8[sl]),
            "wk": _prep_w(Wk[sl]),
            "wv": _prep_w(Wv[sl]),
            "wo": np.ascontiguousarray(
                np.ascontiguousarray(Wo[:, sl].T)  # [512 c', 1024 co]
                .reshape(4, 128, C)
                .transpose(1, 0, 2)
            ).astype(NP_BF16),
            "masks": masks,
        }
        if aug:
            m["waug"] = np.stack([bq8[sl], bk[sl], bv[sl]])[None].astype(NP_BF16)
        in_maps.append(m)

    res = run_bass_kernel_spmd(nc, in_maps, core_ids=list(range(NCORES)))
    y = np.empty((B, T, C), np.float32)
    for b in range(B):
        y[b] = res.results[2 * b]["y"] + res.results[2 * b + 1]["y"]
    y += bo
    return y
